# revision 2
# baseline (speedup 1.0000x reference)
"""Causal dot-product attention (B=4, S=2048, D=1024) on 8 TRN2 NeuronCores.

Sharding: batch x query-tile-class. Core c handles batch c//2; the 16
query row-tiles (128 rows each) of a batch are split between its two
cores so that both get the same padded causal-extent sequence
[4,4,8,8,12,12,16,16] (in 128-key blocks) -> one SPMD program for all
8 cores. k/v/q projection weights are replicated.

Numerics: projections and QK^T run in fp32r (11-bit-mantissa fp32,
full PE speed); softmax stats in fp32 on ACT/DVE; probs and V in bf16
for the SV matmul. The causal mask is applied from the real mask input
via a fused (mask*2^19 + logits) op before max-subtraction.
"""
import numpy as np
import concourse.bass as bass
import concourse.mybir as mybir
from concourse import bacc
from concourse.tile import TileContext
from concourse.bass_utils import run_bass_kernel_spmd
from concourse.masks import make_identity

f32 = mybir.dt.float32
f32r = mybir.dt.float32r
bf16 = mybir.dt.bfloat16
u8 = mybir.dt.uint8
AF = mybir.ActivationFunctionType
ALU = mybir.AluOpType

B, S, D = 4, 2048, 1024
SH = 1024                 # query rows per core
NSLOT = 8                 # 128-row query tiles per core
NBLK = [1, 1, 2, 2, 3, 3, 4, 4]   # padded extent per slot, in 512-key blocks
TILES = [[0, 1, 4, 5, 8, 9, 12, 13], [2, 3, 6, 7, 10, 11, 14, 15]]
MOFF = float(2 ** 19)     # mask offset; >> max |logit| (~1.4e5)
SCALE = 1.0 / 32.0        # 1/sqrt(D)


def build():
    nc = bacc.Bacc()
    qT = nc.dram_tensor("qT", [D, SH], f32, kind="ExternalInput")
    kT = nc.dram_tensor("kT", [D, S], f32, kind="ExternalInput")
    vT = nc.dram_tensor("vT", [D, S], f32, kind="ExternalInput")
    Wq = nc.dram_tensor("Wq", [D, D], f32, kind="ExternalInput")
    Wk = nc.dram_tensor("Wk", [D, D], f32, kind="ExternalInput")
    Wv = nc.dram_tensor("Wv", [D, D], f32, kind="ExternalInput")
    Mu = nc.dram_tensor("Mu", [SH, S], u8, kind="ExternalInput")
    O = nc.dram_tensor("O", [SH, D], f32, kind="ExternalOutput")

    qT3 = qT.rearrange("(c p) n -> p c n", p=128)   # [128, 8, 1024]
    kT3 = kT.rearrange("(c p) n -> p c n", p=128)   # [128, 8, 2048]
    vT3 = vT.rearrange("(c p) n -> p c n", p=128)
    Wq3 = Wq.rearrange("(c p) n -> p c n", p=128)
    Wk3 = Wk.rearrange("(c p) n -> p c n", p=128)
    Wv3 = Wv.rearrange("(c p) n -> p c n", p=128)

    with TileContext(nc) as tc:
        with tc.tile_pool(name="pers", bufs=1) as pers:
            # persistent intermediates
            k1T = pers.tile([128, 8, S], f32r, tag="k1T")      # 64 KB/part
            v1 = pers.tile([128, 16, D], bf16, tag="v1")       # 32 KB/part
            q1T = pers.tile([128, 8, SH], f32r, tag="q1T")     # 32 KB/part
            ident = pers.tile([128, 128], bf16, tag="ident")   # for PE transpose
            make_identity(nc, ident[:])

            # ---- projections (fp32r) ----
            with (
                tc.tile_pool(name="wpool", bufs=1) as wpool,
                tc.tile_pool(name="inp", bufs=2) as inp,
                tc.tile_pool(name="pps", bufs=4, space="PSUM") as pps,
            ):
                # k1T[dout, s] = sum_din Wk[din, dout] * kT[din, s]
                w = wpool.tile([128, 8, D], f32r, tag="W")
                nc.gpsimd.dma_start(out=w, in_=Wk3)
                for sb in range(4):
                    it = inp.tile([128, 8, 512], f32r, tag="inT")
                    nc.gpsimd.dma_start(out=it, in_=kT3[:, :, sb * 512:(sb + 1) * 512])
                    for dout in range(8):
                        ps = pps.tile([128, 512], f32, tag="pp")
                        for din in range(8):
                            nc.tensor.matmul(
                                ps[:], w[:, din, dout * 128:(dout + 1) * 128],
                                it[:, din, :], start=(din == 0), stop=(din == 7))
                        nc.vector.tensor_copy(
                            k1T[:, dout, sb * 512:(sb + 1) * 512], ps[:])

                # q1T[dout, s] = (1/32) * sum_din Wq[din, dout] * qT[din, s]
                w = wpool.tile([128, 8, D], f32r, tag="W")
                nc.gpsimd.dma_start(out=w, in_=Wq3)
                for sb in range(2):
                    it = inp.tile([128, 8, 512], f32r, tag="inT")
                    nc.gpsimd.dma_start(out=it, in_=qT3[:, :, sb * 512:(sb + 1) * 512])
                    for dout in range(8):
                        ps = pps.tile([128, 512], f32, tag="pp")
                        for din in range(8):
                            nc.tensor.matmul(
                                ps[:], w[:, din, dout * 128:(dout + 1) * 128],
                                it[:, din, :], start=(din == 0), stop=(din == 7))
                        nc.vector.tensor_scalar_mul(
                            q1T[:, dout, sb * 512:(sb + 1) * 512], ps[:], SCALE)

                # v1[key, dv] = sum_din vT[din, key] * Wv[din, dv]
                w = wpool.tile([128, 8, D], f32r, tag="W")
                nc.gpsimd.dma_start(out=w, in_=Wv3)
                for sb in range(4):
                    it = inp.tile([128, 8, 512], f32r, tag="inT")
                    nc.gpsimd.dma_start(out=it, in_=vT3[:, :, sb * 512:(sb + 1) * 512])
                    for kc in range(4):
                        for dv in range(2):
                            ps = pps.tile([128, 512], f32, tag="pp")
                            for din in range(8):
                                nc.tensor.matmul(
                                    ps[:], it[:, din, kc * 128:(kc + 1) * 128],
                                    w[:, din, dv * 512:(dv + 1) * 512],
                                    start=(din == 0), stop=(din == 7))
                            nc.vector.tensor_copy(
                                v1[:, sb * 4 + kc, dv * 512:(dv + 1) * 512], ps[:])

            # ---- attention, one 128-row query tile per slot ----
            with (
                tc.tile_pool(name="work", bufs=2) as work,
                tc.tile_pool(name="small", bufs=2) as small,
                tc.tile_pool(name="qkps", bufs=1, space="PSUM") as qkps,
                tc.tile_pool(name="tpps", bufs=2, space="PSUM") as tpps,
                tc.tile_pool(name="svps", bufs=2, space="PSUM") as svps,
            ):
                for s in range(NSLOT):
                    nblk = NBLK[s]
                    E = 4 * nblk              # extent in 128-key chunks
                    L = 512 * nblk            # extent in keys
                    # QK^T logits into PSUM: [128 q, L]
                    qk = qkps.tile([128, 4, 512], f32, tag="qk")
                    for j4 in range(nblk):
                        for din in range(8):
                            nc.tensor.matmul(
                                qk[:, j4, :],
                                q1T[:, din, s * 128:(s + 1) * 128],
                                k1T[:, din, j4 * 512:(j4 + 1) * 512],
                                start=(din == 0), stop=(din == 7))
                    # mask: logits += mask*2^19 (allowed ~2^19, masked stays small)
                    mu = work.tile([128, 4, 512], u8, tag="mu")
                    nc.sync.dma_start(out=mu[:, :nblk, :], in_=Mu[s * 128:(s + 1) * 128, :L])
                    maxs = small.tile([128, 4], f32, tag="maxs")
                    for j4 in range(nblk):
                        nc.vector.scalar_tensor_tensor(
                            qk[:, j4, :], mu[:, j4, :], MOFF, qk[:, j4, :],
                            op0=ALU.mult, op1=ALU.add)
                        nc.vector.tensor_reduce(
                            maxs[:, j4:j4 + 1], qk[:, j4, :],
                            axis=mybir.AxisListType.X, op=ALU.max)
                    negmax = small.tile([128, 1], f32, tag="negmax")
                    nc.vector.tensor_reduce(
                        negmax[:], maxs[:, :nblk], axis=mybir.AxisListType.X,
                        op=ALU.max, negate=True)
                    # exp + per-block row sums
                    probs = work.tile([128, 4, 512], bf16, tag="probs")
                    sums = small.tile([128, 4], f32, tag="sums")
                    for j4 in range(nblk):
                        nc.scalar.activation(
                            probs[:, j4, :], qk[:, j4, :], AF.Exp,
                            bias=negmax[:, 0:1], scale=1.0,
                            accum_out=sums[:, j4:j4 + 1])
                    total = small.tile([128, 1], f32, tag="total")
                    nc.vector.tensor_reduce(
                        total[:], sums[:, :nblk], axis=mybir.AxisListType.X, op=ALU.add)
                    recip = small.tile([128, 1], f32, tag="recip")
                    nc.vector.reciprocal(recip[:], total[:])
                    # transpose probs 128x128 blocks (PE)
                    pT = work.tile([128, 16, 128], bf16, tag="pT")
                    p2 = probs[:].rearrange("p a b -> p (a b)")
                    for j in range(E):
                        tp = tpps.tile([128, 128], bf16, tag="tp")
                        nc.tensor.transpose(tp[:], p2[:, j * 128:(j + 1) * 128], ident[:])
                        nc.vector.tensor_copy(pT[:, j, :], tp[:])
                    # SV: out[q, dv] = sum_j pT[j].T @ v1[j, dv]
                    ot = work.tile([128, D], f32, tag="ot")
                    for dv in range(2):
                        sv = svps.tile([128, 512], f32, tag="sv")
                        for j in range(E):
                            nc.tensor.matmul(
                                sv[:], pT[:, j, :], v1[:, j, dv * 512:(dv + 1) * 512],
                                start=(j == 0), stop=(j == E - 1))
                        # normalize by 1/rowsum during evacuation
                        nc.scalar.activation(
                            ot[:, dv * 512:(dv + 1) * 512], sv[:], AF.Copy,
                            bias=0.0, scale=recip[:, 0:1])
                    nc.sync.dma_start(out=O[s * 128:(s + 1) * 128, :], in_=ot[:])
    nc.finalize()
    return nc


_NC_CACHE = []


def kernel(q, k, v, mask, W_q, W_k, W_v):
    q = np.asarray(q, dtype=np.float32)
    k = np.asarray(k, dtype=np.float32)
    v = np.asarray(v, dtype=np.float32)
    W_q = np.asarray(W_q, dtype=np.float32)
    W_k = np.asarray(W_k, dtype=np.float32)
    W_v = np.asarray(W_v, dtype=np.float32)
    mask_u8 = np.asarray(mask).astype(np.uint8)

    if not _NC_CACHE:
        _NC_CACHE.append(build())
    nc = _NC_CACHE[0]

    row_sets = []
    in_maps = []
    for c in range(8):
        b, cls = c // 2, c % 2
        rows = np.concatenate([np.arange(128 * t, 128 * (t + 1)) for t in TILES[cls]])
        row_sets.append((b, rows))
        in_maps.append({
            "qT": np.ascontiguousarray(q[b][rows, :].T),
            "kT": np.ascontiguousarray(k[b].T),
            "vT": np.ascontiguousarray(v[b].T),
            "Wq": W_q, "Wk": W_k, "Wv": W_v,
            "Mu": np.ascontiguousarray(mask_u8[b][rows, :]),
        })

    res = run_bass_kernel_spmd(nc, in_maps, core_ids=list(range(8)))

    out = np.empty((B, S, D), dtype=np.float32)
    for c in range(8):
        b, rows = row_sets[c]
        out[b][rows, :] = res.results[c]["O"]
    return out


# revision 3
# speedup vs baseline: 1.1736x; 1.1736x over previous
"""Causal dot-product attention (B=4, S=2048, D=1024) on 8 TRN2 NeuronCores.

Sharding: batch x query-tile-class. Core c handles batch c//2; the 16
query row-tiles (128 rows each) of a batch are split between its two
cores so that both get the same padded causal-extent sequence
[4,4,8,8,12,12,16,16] (in 128-key blocks) -> one SPMD program for all
8 cores. k/v/q projection weights are replicated.

Numerics: projections and QK^T run in fp32r (11-bit-mantissa fp32,
full PE speed); softmax stats in fp32 on ACT/DVE; probs and V in bf16
for the SV matmul. The causal mask is applied from the real mask input
via a fused (mask*2^19 + logits) op before max-subtraction.
"""
import numpy as np
import concourse.bass as bass
import concourse.mybir as mybir
from concourse import bacc
from concourse.tile import TileContext
from concourse.bass_utils import run_bass_kernel_spmd
from concourse.masks import make_identity

f32 = mybir.dt.float32
f32r = mybir.dt.float32r
bf16 = mybir.dt.bfloat16
u8 = mybir.dt.uint8
AF = mybir.ActivationFunctionType
ALU = mybir.AluOpType

B, S, D = 4, 2048, 1024
SH = 1024                 # query rows per core
NSLOT = 8                 # 128-row query tiles per core
NBLK = [1, 1, 2, 2, 3, 3, 4, 4]   # padded extent per slot, in 512-key blocks
TILES = [[0, 1, 4, 5, 8, 9, 12, 13], [2, 3, 6, 7, 10, 11, 14, 15]]
MOFF = float(2 ** 19)     # mask offset; >> max |logit| (~1.4e5)
SCALE = 1.0 / 32.0        # 1/sqrt(D)


def _proj_xw(nc, wpool, inp, pps, W3, X3, nsb, out_slice, evac):
    """out[dout, s] = sum_din W[din, dout] * X[din, s], W streamed in halves."""
    for wh in range(2):
        w = wpool.tile([128, 8, 512], f32r, tag="W")
        nc.gpsimd.dma_start(out=w, in_=W3[:, :, wh * 512:(wh + 1) * 512])
        for sb in range(nsb):
            it = inp.tile([128, 8, 512], f32r, tag="inT")
            nc.gpsimd.dma_start(out=it, in_=X3[:, :, sb * 512:(sb + 1) * 512])
            for d4 in range(4):
                dout = wh * 4 + d4
                ps = pps.tile([128, 512], f32, tag="pp")
                for din in range(8):
                    nc.tensor.matmul(
                        ps[:], w[:, din, d4 * 128:(d4 + 1) * 128],
                        it[:, din, :], start=(din == 0), stop=(din == 7))
                evac(out_slice(dout, sb), ps)


def build():
    nc = bacc.Bacc()
    qT = nc.dram_tensor("qT", [D, SH], f32, kind="ExternalInput")
    kT = nc.dram_tensor("kT", [D, S], f32, kind="ExternalInput")
    vT = nc.dram_tensor("vT", [D, S], f32, kind="ExternalInput")
    Wq = nc.dram_tensor("Wq", [D, D], f32, kind="ExternalInput")
    Wk = nc.dram_tensor("Wk", [D, D], f32, kind="ExternalInput")
    Wv = nc.dram_tensor("Wv", [D, D], f32, kind="ExternalInput")
    Mu = nc.dram_tensor("Mu", [SH, S], u8, kind="ExternalInput")
    O = nc.dram_tensor("O", [SH, D], f32, kind="ExternalOutput")

    qT3 = qT.rearrange("(c p) n -> p c n", p=128)   # [128, 8, 1024]
    kT3 = kT.rearrange("(c p) n -> p c n", p=128)   # [128, 8, 2048]
    vT3 = vT.rearrange("(c p) n -> p c n", p=128)
    Wq3 = Wq.rearrange("(c p) n -> p c n", p=128)
    Wk3 = Wk.rearrange("(c p) n -> p c n", p=128)
    Wv3 = Wv.rearrange("(c p) n -> p c n", p=128)

    with TileContext(nc) as tc:
        with tc.tile_pool(name="pers", bufs=1) as pers:
            # persistent intermediates
            k1T = pers.tile([128, 8, S], f32r, tag="k1T")      # 64 KB/part
            v1 = pers.tile([128, 16, D], bf16, tag="v1")       # 32 KB/part
            q1T = pers.tile([128, 8, SH], f32r, tag="q1T")     # 32 KB/part
            ident = pers.tile([128, 128], bf16, tag="ident")   # for PE transpose

            # ---- projections (fp32r) ----
            with (
                tc.tile_pool(name="wpool", bufs=2) as wpool,
                tc.tile_pool(name="inp", bufs=2) as inp,
                tc.tile_pool(name="pps", bufs=4, space="PSUM") as pps,
            ):
                # k1T[dout, s] = sum_din Wk[din, dout] * kT[din, s]
                _proj_xw(nc, wpool, inp, pps, Wk3, kT3, 4,
                         lambda dout, sb: k1T[:, dout, sb * 512:(sb + 1) * 512],
                         lambda dst, ps: nc.vector.tensor_copy(dst, ps[:]))
                make_identity(nc, ident[:])
                # q1T[dout, s] = (1/32) * sum_din Wq[din, dout] * qT[din, s]
                _proj_xw(nc, wpool, inp, pps, Wq3, qT3, 2,
                         lambda dout, sb: q1T[:, dout, sb * 512:(sb + 1) * 512],
                         lambda dst, ps: nc.vector.tensor_scalar_mul(dst, ps[:], SCALE))
                # v1[key, dv] = sum_din vT[din, key] * Wv[din, dv]
                for dv in range(2):
                    w = wpool.tile([128, 8, 512], f32r, tag="W")
                    nc.gpsimd.dma_start(out=w, in_=Wv3[:, :, dv * 512:(dv + 1) * 512])
                    for sb in range(4):
                        it = inp.tile([128, 8, 512], f32r, tag="inT")
                        nc.gpsimd.dma_start(out=it, in_=vT3[:, :, sb * 512:(sb + 1) * 512])
                        for kc in range(4):
                            ps = pps.tile([128, 512], f32, tag="pp")
                            for din in range(8):
                                nc.tensor.matmul(
                                    ps[:], it[:, din, kc * 128:(kc + 1) * 128],
                                    w[:, din, :], start=(din == 0), stop=(din == 7))
                            nc.vector.tensor_copy(
                                v1[:, sb * 4 + kc, dv * 512:(dv + 1) * 512], ps[:])

            # ---- attention, one 128-row query tile per slot ----
            with (
                tc.tile_pool(name="work", bufs=2) as work,
                tc.tile_pool(name="small", bufs=2) as small,
                tc.tile_pool(name="qkps", bufs=3, space="PSUM") as qkps,
                tc.tile_pool(name="tpps", bufs=2, space="PSUM") as tpps,
                tc.tile_pool(name="svps", bufs=3, space="PSUM") as svps,
            ):
                for s in range(NSLOT):
                    nblk = NBLK[s]
                    E = 4 * nblk              # extent in 128-key chunks
                    L = 512 * nblk            # extent in keys
                    mu = work.tile([128, 4, 512], u8, tag="mu")
                    nc.sync.dma_start(out=mu[:, :nblk, :], in_=Mu[s * 128:(s + 1) * 128, :L])
                    logits = work.tile([128, 4, 512], f32, tag="lg")
                    maxs = small.tile([128, 4], f32, tag="maxs")
                    for j4 in range(nblk):
                        qk = qkps.tile([128, 512], f32, tag="qk")
                        for din in range(8):
                            nc.tensor.matmul(
                                qk[:],
                                q1T[:, din, s * 128:(s + 1) * 128],
                                k1T[:, din, j4 * 512:(j4 + 1) * 512],
                                start=(din == 0), stop=(din == 7))
                        # logits = mask*2^19 + qk  (allowed ~2^19, masked small)
                        nc.vector.scalar_tensor_tensor(
                            logits[:, j4, :], mu[:, j4, :], MOFF, qk[:],
                            op0=ALU.mult, op1=ALU.add)
                        nc.vector.tensor_reduce(
                            maxs[:, j4:j4 + 1], logits[:, j4, :],
                            axis=mybir.AxisListType.X, op=ALU.max)
                    negmax = small.tile([128, 1], f32, tag="negmax")
                    nc.vector.tensor_reduce(
                        negmax[:], maxs[:, :nblk], axis=mybir.AxisListType.X,
                        op=ALU.max, negate=True)
                    # exp + per-block row sums
                    probs = work.tile([128, 4, 512], bf16, tag="probs")
                    sums = small.tile([128, 4], f32, tag="sums")
                    for j4 in range(nblk):
                        nc.scalar.activation(
                            probs[:, j4, :], logits[:, j4, :], AF.Exp,
                            bias=negmax[:, 0:1], scale=1.0,
                            accum_out=sums[:, j4:j4 + 1])
                    total = small.tile([128, 1], f32, tag="total")
                    nc.vector.tensor_reduce(
                        total[:], sums[:, :nblk], axis=mybir.AxisListType.X, op=ALU.add)
                    recip = small.tile([128, 1], f32, tag="recip")
                    nc.vector.reciprocal(recip[:], total[:])
                    # transpose probs 128x128 blocks (PE)
                    pT = work.tile([128, 16, 128], bf16, tag="pT")
                    p2 = probs[:].rearrange("p a b -> p (a b)")
                    for j in range(E):
                        tp = tpps.tile([128, 128], bf16, tag="tp")
                        nc.tensor.transpose(tp[:], p2[:, j * 128:(j + 1) * 128], ident[:])
                        nc.vector.tensor_copy(pT[:, j, :], tp[:])
                    # SV: out[q, dv] = sum_j pT[j].T @ v1[j, dv]
                    ot = work.tile([128, D], f32, tag="ot")
                    for dv in range(2):
                        sv = svps.tile([128, 512], f32, tag="sv")
                        for j in range(E):
                            nc.tensor.matmul(
                                sv[:], pT[:, j, :], v1[:, j, dv * 512:(dv + 1) * 512],
                                start=(j == 0), stop=(j == E - 1))
                        # normalize by 1/rowsum during evacuation
                        nc.scalar.activation(
                            ot[:, dv * 512:(dv + 1) * 512], sv[:], AF.Copy,
                            bias=0.0, scale=recip[:, 0:1])
                    nc.sync.dma_start(out=O[s * 128:(s + 1) * 128, :], in_=ot[:])
    nc.finalize()
    return nc


_NC_CACHE = []


def kernel(q, k, v, mask, W_q, W_k, W_v):
    q = np.asarray(q, dtype=np.float32)
    k = np.asarray(k, dtype=np.float32)
    v = np.asarray(v, dtype=np.float32)
    W_q = np.asarray(W_q, dtype=np.float32)
    W_k = np.asarray(W_k, dtype=np.float32)
    W_v = np.asarray(W_v, dtype=np.float32)
    mask_u8 = np.asarray(mask).astype(np.uint8)

    if not _NC_CACHE:
        _NC_CACHE.append(build())
    nc = _NC_CACHE[0]

    row_sets = []
    in_maps = []
    for c in range(8):
        b, cls = c // 2, c % 2
        rows = np.concatenate([np.arange(128 * t, 128 * (t + 1)) for t in TILES[cls]])
        row_sets.append((b, rows))
        in_maps.append({
            "qT": np.ascontiguousarray(q[b][rows, :].T),
            "kT": np.ascontiguousarray(k[b].T),
            "vT": np.ascontiguousarray(v[b].T),
            "Wq": W_q, "Wk": W_k, "Wv": W_v,
            "Mu": np.ascontiguousarray(mask_u8[b][rows, :]),
        })

    res = run_bass_kernel_spmd(nc, in_maps, core_ids=list(range(8)))

    out = np.empty((B, S, D), dtype=np.float32)
    for c in range(8):
        b, rows = row_sets[c]
        out[b][rows, :] = res.results[c]["O"]
    return out


# revision 5
# speedup vs baseline: 1.3587x; 1.1577x over previous
"""Causal dot-product attention (B=4, S=2048, D=1024) on 8 TRN2 NeuronCores.

Sharding: batch x query-tile-class. Core c handles batch c//2; the 16
query row-tiles (128 rows each) of a batch are split between its two
cores so that both get the same padded causal-extent sequence (in
512-key blocks, descending) [4,4,3,3,2,2,1,1] -> one SPMD program for
all 8 cores. Projection weights are replicated.

Numerics: projections and QK^T run in fp32r (11-bit-mantissa fp32,
full PE speed); softmax stats in fp32 on ACT/DVE; probs and V in bf16
for the SV matmul. The causal mask is applied from the real mask input
via a fused (mask*2^19 + logits) op before max-subtraction.
"""
import numpy as np
import concourse.bass as bass
import concourse.mybir as mybir
from concourse import bacc
from concourse.tile import TileContext
from concourse.bass_utils import run_bass_kernel_spmd
from concourse.masks import make_identity

f32 = mybir.dt.float32
f32r = mybir.dt.float32r
bf16 = mybir.dt.bfloat16
u8 = mybir.dt.uint8
AF = mybir.ActivationFunctionType
ALU = mybir.AluOpType

B, S, D = 4, 2048, 1024
SH = 1024                 # query rows per core
NSLOT = 8                 # 128-row query tiles per core
NBLK = [4, 4, 3, 3, 2, 2, 1, 1]   # padded extent per slot, in 512-key blocks
TILES = [[12, 13, 8, 9, 4, 5, 0, 1], [14, 15, 10, 11, 6, 7, 2, 3]]
MOFF = float(2 ** 19)     # mask offset; >> max |logit| (~1.4e5)
SCALE = 1.0 / 32.0        # 1/sqrt(D)


def build():
    nc = bacc.Bacc()
    qT = nc.dram_tensor("qT", [D, SH], f32, kind="ExternalInput")
    kT = nc.dram_tensor("kT", [D, S], f32, kind="ExternalInput")
    vT = nc.dram_tensor("vT", [D, S], f32, kind="ExternalInput")
    Wq = nc.dram_tensor("Wq", [D, D], f32, kind="ExternalInput")
    Wk = nc.dram_tensor("Wk", [D, D], f32, kind="ExternalInput")
    Wv = nc.dram_tensor("Wv", [D, D], f32, kind="ExternalInput")
    Mu = nc.dram_tensor("Mu", [SH, S], u8, kind="ExternalInput")
    O = nc.dram_tensor("O", [SH, D], f32, kind="ExternalOutput")

    qT3 = qT.rearrange("(c p) n -> p c n", p=128)   # [128, 8, 1024]
    kT3 = kT.rearrange("(c p) n -> p c n", p=128)   # [128, 8, 2048]
    vT3 = vT.rearrange("(c p) n -> p c n", p=128)
    Wq3 = Wq.rearrange("(c p) n -> p c n", p=128)
    Wk3 = Wk.rearrange("(c p) n -> p c n", p=128)
    Wv3 = Wv.rearrange("(c p) n -> p c n", p=128)

    def load_w(pool, W3):
        w = pool.tile([128, 8, D], f32r, tag="W")
        # two half-DMAs so the first douts can start sooner
        nc.gpsimd.dma_start(out=w[:, :, 0:512], in_=W3[:, :, 0:512])
        nc.gpsimd.dma_start(out=w[:, :, 512:1024], in_=W3[:, :, 512:1024])
        return w

    with TileContext(nc) as tc:
        with tc.tile_pool(name="pers", bufs=1) as pers:
            k1T = pers.tile([128, 8, S], f32r, tag="k1T")      # 64 KB/part
            v1 = pers.tile([128, 16, D], bf16, tag="v1")       # 32 KB/part

            inp = tc.alloc_tile_pool(name="inp", bufs=2, side="left")
            wk_pool = tc.alloc_tile_pool(name="wk", bufs=1, side="left")
            wv_pool = tc.alloc_tile_pool(name="wv", bufs=1, side="right")

            with tc.tile_pool(name="pps", bufs=4, space="PSUM") as pps:
                # ---- k1T[dout, s] = sum_din Wk[din, dout] * kT[din, s] ----
                wk = load_w(wk_pool, Wk3)
                wv = None
                for sb in range(4):
                    it = inp.tile([128, 8, 512], f32r, tag="inT")
                    nc.gpsimd.dma_start(out=it, in_=kT3[:, :, sb * 512:(sb + 1) * 512])
                    for dout in range(8):
                        ps = pps.tile([128, 512], f32, tag="pp")
                        for din in range(8):
                            nc.tensor.matmul(
                                ps[:], wk[:, din, dout * 128:(dout + 1) * 128],
                                it[:, din, :], start=(din == 0), stop=(din == 7))
                        nc.vector.tensor_copy(
                            k1T[:, dout, sb * 512:(sb + 1) * 512], ps[:])
                    if sb == 1:
                        wv = load_w(wv_pool, Wv3)   # preload during k1
                wk_pool.release()
                wq_pool = tc.alloc_tile_pool(name="wq", bufs=1, side="left")

                # ---- v1[key, dv] = sum_din vT[din, key] * Wv[din, dv] ----
                wq = None
                for sb in range(4):
                    it = inp.tile([128, 8, 512], f32r, tag="inT")
                    nc.gpsimd.dma_start(out=it, in_=vT3[:, :, sb * 512:(sb + 1) * 512])
                    for kc in range(4):
                        for dv in range(2):
                            ps = pps.tile([128, 512], f32, tag="pp")
                            for din in range(8):
                                nc.tensor.matmul(
                                    ps[:], it[:, din, kc * 128:(kc + 1) * 128],
                                    wv[:, din, dv * 512:(dv + 1) * 512],
                                    start=(din == 0), stop=(din == 7))
                            nc.vector.tensor_copy(
                                v1[:, sb * 4 + kc, dv * 512:(dv + 1) * 512], ps[:])
                    if sb == 1:
                        wq = load_w(wq_pool, Wq3)   # preload during v1
                wv_pool.release()
                q1_pool = tc.alloc_tile_pool(name="q1p", bufs=1, side="right")
                q1T = q1_pool.tile([128, 8, SH], f32r, tag="q1T")  # 32 KB/part

                # ---- q1T[dout, s] = (1/32) * sum_din Wq[din, dout] * qT[din, s] ----
                for sb in range(2):
                    it = inp.tile([128, 8, 512], f32r, tag="inT")
                    nc.gpsimd.dma_start(out=it, in_=qT3[:, :, sb * 512:(sb + 1) * 512])
                    for dout in range(8):
                        ps = pps.tile([128, 512], f32, tag="pp")
                        for din in range(8):
                            nc.tensor.matmul(
                                ps[:], wq[:, din, dout * 128:(dout + 1) * 128],
                                it[:, din, :], start=(din == 0), stop=(din == 7))
                        nc.vector.tensor_scalar_mul(
                            q1T[:, dout, sb * 512:(sb + 1) * 512], ps[:], SCALE)
                wq_pool.release()
                inp.release()

            # ---- attention, one 128-row query tile per slot ----
            with (
                tc.tile_pool(name="work", bufs=2) as work,
                tc.tile_pool(name="small", bufs=2) as small,
                tc.tile_pool(name="qkps", bufs=3, space="PSUM") as qkps,
                tc.tile_pool(name="tpps", bufs=2, space="PSUM") as tpps,
                tc.tile_pool(name="svps", bufs=3, space="PSUM") as svps,
            ):
                ident = work.tile([128, 128], bf16, tag="ident")
                make_identity(nc, ident[:])
                for s in range(NSLOT):
                    nblk = NBLK[s]
                    E = 4 * nblk              # extent in 128-key chunks
                    L = 512 * nblk            # extent in keys
                    mu = work.tile([128, 4, 512], u8, tag="mu")
                    nc.sync.dma_start(out=mu[:, :nblk, :], in_=Mu[s * 128:(s + 1) * 128, :L])
                    logits = work.tile([128, 4, 512], f32, tag="lg")
                    maxs = small.tile([128, 4], f32, tag="maxs")
                    for j4 in range(nblk):
                        qk = qkps.tile([128, 512], f32, tag="qk")
                        for din in range(8):
                            nc.tensor.matmul(
                                qk[:],
                                q1T[:, din, s * 128:(s + 1) * 128],
                                k1T[:, din, j4 * 512:(j4 + 1) * 512],
                                start=(din == 0), stop=(din == 7))
                        # logits = mask*2^19 + qk  (allowed ~2^19, masked small)
                        nc.vector.scalar_tensor_tensor(
                            logits[:, j4, :], mu[:, j4, :], MOFF, qk[:],
                            op0=ALU.mult, op1=ALU.add)
                        nc.vector.tensor_reduce(
                            maxs[:, j4:j4 + 1], logits[:, j4, :],
                            axis=mybir.AxisListType.X, op=ALU.max)
                    negmax = small.tile([128, 1], f32, tag="negmax")
                    nc.vector.tensor_reduce(
                        negmax[:], maxs[:, :nblk], axis=mybir.AxisListType.X,
                        op=ALU.max, negate=True)
                    # exp + per-block row sums
                    probs = work.tile([128, 4, 512], bf16, tag="probs")
                    sums = small.tile([128, 4], f32, tag="sums")
                    for j4 in range(nblk):
                        nc.scalar.activation(
                            probs[:, j4, :], logits[:, j4, :], AF.Exp,
                            bias=negmax[:, 0:1], scale=1.0,
                            accum_out=sums[:, j4:j4 + 1])
                    total = small.tile([128, 1], f32, tag="total")
                    nc.vector.tensor_reduce(
                        total[:], sums[:, :nblk], axis=mybir.AxisListType.X, op=ALU.add)
                    recip = small.tile([128, 1], f32, tag="recip")
                    nc.vector.reciprocal(recip[:], total[:])
                    # transpose probs 128x128 blocks (PE)
                    pT = work.tile([128, 16, 128], bf16, tag="pT")
                    p2 = probs[:].rearrange("p a b -> p (a b)")
                    for j in range(E):
                        tp = tpps.tile([128, 128], bf16, tag="tp")
                        nc.tensor.transpose(tp[:], p2[:, j * 128:(j + 1) * 128], ident[:])
                        nc.vector.tensor_copy(pT[:, j, :], tp[:])
                    # SV: out[q, dv] = sum_j pT[j].T @ v1[j, dv]
                    ot = work.tile([128, D], f32, tag="ot")
                    for dv in range(2):
                        sv = svps.tile([128, 512], f32, tag="sv")
                        for j in range(E):
                            nc.tensor.matmul(
                                sv[:], pT[:, j, :], v1[:, j, dv * 512:(dv + 1) * 512],
                                start=(j == 0), stop=(j == E - 1))
                        # normalize by 1/rowsum during evacuation
                        nc.scalar.activation(
                            ot[:, dv * 512:(dv + 1) * 512], sv[:], AF.Copy,
                            bias=0.0, scale=recip[:, 0:1])
                    nc.sync.dma_start(out=O[s * 128:(s + 1) * 128, :], in_=ot[:])
            q1_pool.release()
    nc.finalize()
    return nc


_NC_CACHE = []


def kernel(q, k, v, mask, W_q, W_k, W_v):
    q = np.asarray(q, dtype=np.float32)
    k = np.asarray(k, dtype=np.float32)
    v = np.asarray(v, dtype=np.float32)
    W_q = np.asarray(W_q, dtype=np.float32)
    W_k = np.asarray(W_k, dtype=np.float32)
    W_v = np.asarray(W_v, dtype=np.float32)
    mask_u8 = np.asarray(mask).astype(np.uint8)

    if not _NC_CACHE:
        _NC_CACHE.append(build())
    nc = _NC_CACHE[0]

    row_sets = []
    in_maps = []
    for c in range(8):
        b, cls = c // 2, c % 2
        rows = np.concatenate([np.arange(128 * t, 128 * (t + 1)) for t in TILES[cls]])
        row_sets.append((b, rows))
        in_maps.append({
            "qT": np.ascontiguousarray(q[b][rows, :].T),
            "kT": np.ascontiguousarray(k[b].T),
            "vT": np.ascontiguousarray(v[b].T),
            "Wq": W_q, "Wk": W_k, "Wv": W_v,
            "Mu": np.ascontiguousarray(mask_u8[b][rows, :]),
        })

    res = run_bass_kernel_spmd(nc, in_maps, core_ids=list(range(8)))

    out = np.empty((B, S, D), dtype=np.float32)
    for c in range(8):
        b, rows = row_sets[c]
        out[b][rows, :] = res.results[c]["O"]
    return out
